# revision 1
# baseline (speedup 1.0000x reference)
"""Causal self-attention (RMSNorm + QKV + causal attention + out-proj) on 8 trn2
NeuronCores.

Sharding: core c handles batch b=c//2 and head-group g=c%2 (Megatron-style TP
over the 16 heads: 8 heads per group). Each core computes a partial output
y_part[b,g] = attn_out(heads of g) @ w_proj[:, g-cols].T ; the host sums the two
TP partials per batch.

On-core pipeline (T=1024, E=1024, 8 local heads, D=64), all matmuls in bf16
with fp32 PSUM accumulation:
  1. r[t] = 1/sqrt(mean_e x^2 + eps) from x^T tiles (ACT squares + DVE adds +
     ones-matmul partition reduce), broadcast via GPSIMD.
  2. qk^T[f,t] = (Wqk x^T) * r[t]  (RMSNorm folded into the eviction scale);
     V[t,f] = (x W_v^T) * r[t], stored with a ones-column per head (65 cols) so
     the attention matmul also produces softmax denominators.
  3. Per head: S^T tiles = K^T.T @ Q^T (contraction d=64), P = exp(S/8) with a
     0/1 band mask, O^T[d,tq] (+denominator row) = V'.T @ P, normalized at
     eviction by the GPSIMD-broadcast reciprocal denominator row.
  4. y[t,:] partial = sum_h O_h^T.T @ wproj_h rows, DMA'd out in fp32.
"""

import sys

sys.path.insert(0, "/opt/trn_rl_repo")

import numpy as np
import ml_dtypes

import concourse.bass as bass
import concourse.tile as tile
from concourse import mybir
from concourse.bass_utils import run_bass_kernel_spmd

BF16 = ml_dtypes.bfloat16

B, T, E, H = 4, 1024, 1024, 16
D = E // H  # 64
HL = 8  # heads per core (TP degree 2)
FL = HL * D  # 512 local head cols
EPS = 1e-5
N_CORES = 8

AF = mybir.ActivationFunctionType
DT = mybir.dt


# ---------------------------------------------------------------------------
# Walrus in this toolchain rejects instructions whose tail Drain carries more
# than one semaphore wait; split the TileContext exit drain into chained
# single-wait drains.
def _patched_drain_and_barrier(self, tick_clock, wait_clock):
    nc = self.nc
    drain_inst = nc.sync.drain()
    wait_clock.add_sem_waits(
        drain_inst.ins, tile.ScopedClock({None: tick_clock.global_clock})
    )
    mi = drain_inst.ins
    si = mi.sync_info
    if si is not None and len(si.on_wait) > 1:
        waits = list(si.on_wait)
        mi.sync_info = mybir.SyncInfo(on_wait=waits[:1], on_update=list(si.on_update))
        for w in waits[1:]:
            extra = nc.sync.drain().ins
            extra.sync_info = mybir.SyncInfo(on_wait=[w], on_update=[])
    nc.all_engine_barrier()
    assert self.sems is not None
    popped = nc._tile_sem_poison_stack.pop()
    assert popped is self._sem_poison
    nc.clear_and_free_semaphores(list(self.sems.allocated().values()))
    nc.all_engine_barrier()


tile.TileContext._drain_and_barrier = _patched_drain_and_barrier

# The same 1-wait-per-instruction walrus limit applies to every engine's
# instructions. Legalize at BIR-serialization time: hoist all but the last
# wait of a multi-wait instruction onto fresh single-wait Drains inserted
# just before it on the same engine.
_orig_to_json_bytes = bass.Bass.to_json_bytes


def _legalized_to_json_bytes(self):
    import orjson

    j = orjson.loads(_orig_to_json_bytes(self))
    ctr = 0
    for fn in j["functions"]:
        for bb in fn["blocks"]:
            new_insts = []
            for ins in bb["instructions"]:
                si = ins.get("sync_info")
                waits = si.get("on_wait") if si else None
                if waits and len(waits) > 1:
                    for w in waits[:-1]:
                        ctr += 1
                        new_insts.append(
                            {
                                "debug": ins.get("debug"),
                                "engine": ins["engine"],
                                "ins": [],
                                "outs": [],
                                "name": f"I-wf{ctr}",
                                "opcode": "EventSemaphore",
                                "sync_info": {"on_update": [], "on_wait": [w]},
                            }
                        )
                    si["on_wait"] = [waits[-1]]
                new_insts.append(ins)
            bb["instructions"] = new_insts
    return orjson.dumps(j)


bass.Bass.to_json_bytes = _legalized_to_json_bytes


def build_program():
    nc = bass.Bass("TRN2", target_bir_lowering=False, debug=False)

    xt_d = nc.declare_dram_parameter("xt", [E, T], DT.bfloat16, isOutput=False)
    wqk_d = nc.declare_dram_parameter("wqk", [E, 2 * FL], DT.bfloat16, isOutput=False)
    wv_d = nc.declare_dram_parameter("wv", [E, FL], DT.bfloat16, isOutput=False)
    wproj_d = nc.declare_dram_parameter("wproj", [FL, E], DT.bfloat16, isOutput=False)
    mask_d = nc.declare_dram_parameter("mask", [4, 128, 512], DT.bfloat16, isOutput=False)
    y_d = nc.declare_dram_parameter("y", [T, E], DT.float32, isOutput=True)
    import os
    dbg = os.environ.get("KDBG") == "1"
    if dbg:
        dqk_d = nc.declare_dram_parameter("dqk", [8, 128, T], DT.float32, isOutput=True)
        dvp_d = nc.declare_dram_parameter("dvp", [8, 128, HL * 65], DT.float32, isOutput=True)
        dot_d = nc.declare_dram_parameter("dot", [HL, 64, T], DT.float32, isOutput=True)

    NT = T // 128  # 8 tiles of 128

    with tile.TileContext(nc) as tc:
        with (
            tc.tile_pool(name="persist", bufs=1) as persist,
            tc.tile_pool(name="work", bufs=3) as work,
        ):
            # ---- persistent SBUF tensors -------------------------------
            qk_sb = [persist.tile([128, T], DT.bfloat16, tag=f"qk{i}", name=f"qk{i}") for i in range(8)]
            vp_sb = [persist.tile([128, HL * 65], DT.bfloat16, tag=f"vp{i}", name=f"vp{i}") for i in range(NT)]
            ot_sb = [persist.tile([64, T], DT.bfloat16, tag=f"ot{h}", name=f"ot{h}") for h in range(HL)]
            wproj_sb = [persist.tile([64, E], DT.bfloat16, tag=f"wp{h}", name=f"wp{h}") for h in range(HL)]
            mask_sb = [persist.tile([128, 512], DT.bfloat16, tag=f"mk{k}", name=f"mk{k}") for k in range(4)]
            r_bcast = persist.tile([128, T], DT.float32, tag="r_bcast", name="r_bcast")
            ones_col = persist.tile([128, 1], DT.float32, tag="ones_col", name="ones_col")
            r_sb = persist.tile([1, T], DT.float32, tag="r_sb", name="r_sb")
            s_sb = persist.tile([1, T], DT.float32, tag="s_sb", name="s_sb")
            eps_t = persist.tile([1, 1], DT.float32, tag="eps_t", name="eps_t")
            ones_row = persist.tile([128, 128], DT.float32, tag="ones_row", name="ones_row")
            nc.vector.memset(eps_t, float(EPS))
            nc.vector.memset(ones_row, 1.0)

            nc.vector.memset(ones_col, 1.0)
            for h in range(HL):
                nc.gpsimd.dma_start(out=wproj_sb[h], in_=wproj_d[h * 64 : (h + 1) * 64, :])
            for k in range(4):
                nc.gpsimd.dma_start(out=mask_sb[k], in_=mask_d[k])

            # ---- stage A: norm stats + QKV + V -------------------------
            with (
                tc.tile_pool(name="xw", bufs=1) as xw,
                tc.tile_pool(name="psA", bufs=2, space="PSUM") as psA,
                tc.tile_pool(name="psS", bufs=1, space="PSUM") as psS,
            ):
                xt_sb = [xw.tile([128, T], DT.bfloat16, tag=f"xt{i}", name=f"xt{i}") for i in range(8)]
                wqk_sb = [xw.tile([128, 2 * FL], DT.bfloat16, tag=f"wqk{i}", name=f"wqk{i}") for i in range(8)]
                wv_sb = [xw.tile([128, FL], DT.bfloat16, tag=f"wv{i}", name=f"wv{i}") for i in range(8)]
                for i in range(8):
                    nc.gpsimd.dma_start(out=xt_sb[i], in_=xt_d[i * 128 : (i + 1) * 128, :])
                    nc.gpsimd.dma_start(out=wqk_sb[i], in_=wqk_d[i * 128 : (i + 1) * 128, :])
                    nc.gpsimd.dma_start(out=wv_sb[i], in_=wv_d[i * 128 : (i + 1) * 128, :])

                # sum of squares over e (accumulate across the 8 e-tiles)
                sq = [xw.tile([128, T], DT.float32, tag=f"sq{i}", name=f"sq{i}") for i in range(8)]
                for i in range(8):
                    nc.scalar.square(sq[i], xt_sb[i])
                # pairwise tree: 4 + 2 + 1 adds
                for step in (1, 2, 4):
                    for i in range(0, 8, 2 * step):
                        nc.vector.tensor_add(sq[i], sq[i], sq[i + step])
                acc = sq[0]

                sumsq = psS.tile([1, T], DT.float32, tag="sumsq", name="sumsq")
                nc.tensor.matmul(sumsq[0:1, 0:512], ones_col, acc[:, 0:512], start=True, stop=True)
                nc.tensor.matmul(sumsq[0:1, 512:1024], ones_col, acc[:, 512:1024], start=True, stop=True)
                nc.scalar.activation(s_sb, sumsq, AF.Sqrt, bias=eps_t, scale=1.0 / E)
                nc.vector.reciprocal(r_sb, s_sb)
                # broadcast r to all 128 partitions via a K=1 ones-matmul
                for n in range(2):
                    rbp = psS.tile([128, 512], DT.float32, tag="rbp", name="rbp")
                    nc.tensor.matmul(
                        rbp,
                        ones_row[0:1, :],
                        r_sb[0:1, n * 512 : (n + 1) * 512],
                        start=True,
                        stop=True,
                    )
                    nc.vector.tensor_copy(r_bcast[:, n * 512 : (n + 1) * 512], rbp)
                # normalized x tiles for the V path (r folded in)
                xn_sb = [
                    xw.tile([128, T], DT.bfloat16, tag=f"xn{i}", name=f"xn{i}")
                    for i in range(8)
                ]
                for i in range(8):
                    nc.vector.tensor_mul(xn_sb[i], xt_sb[i], r_bcast)

                # qk^T tiles: [f=128, t] = sum_e wqk_e.T @ xt_e, scaled by r[t]
                for fi in range(8):
                    for n in range(2):
                        ps = psA.tile([128, 512], DT.float32, tag="qkps", name="qkps")
                        for ei in range(8):
                            nc.tensor.matmul(
                                ps,
                                wqk_sb[ei][:, fi * 128 : (fi + 1) * 128],
                                xt_sb[ei][:, n * 512 : (n + 1) * 512],
                                start=(ei == 0),
                                stop=(ei == 7),
                            )
                        nc.vector.tensor_mul(
                            qk_sb[fi][:, n * 512 : (n + 1) * 512],
                            ps,
                            r_bcast[:, n * 512 : (n + 1) * 512],
                        )

                # V natural: [t=128, 512] = sum_e xt_e(cols ti).T @ wv_e, * r[t]
                for ti in range(NT):
                    ps = psA.tile([128, 512], DT.float32, tag="vps", name="vps")
                    for ei in range(8):
                        nc.tensor.matmul(
                            ps,
                            xn_sb[ei][:, ti * 128 : (ti + 1) * 128],
                            wv_sb[ei],
                            start=(ei == 0),
                            stop=(ei == 7),
                        )
                    for h in range(HL):
                        nc.vector.tensor_copy(
                            vp_sb[ti][:, h * 65 : h * 65 + 64],
                            ps[:, h * 64 : (h + 1) * 64],
                        )
                        nc.vector.memset(vp_sb[ti][:, h * 65 + 64 : h * 65 + 65], 1.0)

            if dbg:
                with tc.tile_pool(name="dbgp", bufs=2) as dbgp:
                    for i in range(8):
                        dt1 = dbgp.tile([128, T], DT.float32, tag="dt1", name="dt1")
                        nc.vector.tensor_copy(dt1, qk_sb[i])
                        nc.gpsimd.dma_start(out=dqk_d[i], in_=dt1)
                        dt2 = dbgp.tile([128, HL * 65], DT.float32, tag="dt2", name="dt2")
                        nc.vector.tensor_copy(dt2, vp_sb[i])
                        nc.gpsimd.dma_start(out=dvp_d[i], in_=dt2)

            # ---- stage B: attention ------------------------------------
            with (
                tc.tile_pool(name="pP", bufs=14) as pP,
                tc.tile_pool(name="pInv", bufs=3) as pInv,
                tc.tile_pool(name="psST", bufs=3, space="PSUM") as psST,
                tc.tile_pool(name="psOT", bufs=2, space="PSUM") as psOT,
            ):
                for h in range(HL):
                    base = 64 * (h % 2)
                    qt = qk_sb[h // 2][base : base + 64, :]
                    kt = qk_sb[4 + h // 2][base : base + 64, :]
                    for b in range(2):
                        nj = 4 * b + 4
                        ptiles = []
                        for j in range(nj):
                            st = psST.tile([128, 512], DT.float32, tag="st", name="st")
                            nc.tensor.matmul(
                                st,
                                kt[:, j * 128 : (j + 1) * 128],
                                qt[:, b * 512 : (b + 1) * 512],
                                start=True,
                                stop=True,
                            )
                            p_t = pP.tile([128, 512], DT.bfloat16, tag="p_t", name="p_t")
                            if j >= 4 * b:  # diagonal band: mask after exp
                                praw = pP.tile([128, 512], DT.bfloat16, tag="praw", name="praw")
                                nc.scalar.activation(praw, st, AF.Exp, scale=0.125)
                                nc.vector.tensor_mul(p_t, praw, mask_sb[j - 4 * b])
                            else:
                                nc.scalar.activation(p_t, st, AF.Exp, scale=0.125)
                            ptiles.append(p_t)

                        ot = psOT.tile([65, 512], DT.float32, tag="ot", name="ot")
                        for j in range(nj):
                            nc.tensor.matmul(
                                ot,
                                vp_sb[j][:, h * 65 : (h + 1) * 65],
                                ptiles[j],
                                start=(j == 0),
                                stop=(j == nj - 1),
                            )
                        invrow = pInv.tile([65, 512], DT.float32, tag="invrow", name="invrow")
                        nc.vector.reciprocal(invrow[64:65, :], ot[64:65, :])
                        # broadcast 1/denominator to 64 partitions (K=1 matmul)
                        ibp = psST.tile([64, 512], DT.float32, tag="ibp", name="ibp")
                        nc.tensor.matmul(
                            ibp,
                            ones_row[64:65, 0:64],
                            invrow[64:65, :],
                            start=True,
                            stop=True,
                        )
                        invb = pInv.tile([64, 512], DT.float32, tag="invb", name="invb")
                        nc.vector.tensor_copy(invb, ibp)
                        nc.vector.tensor_mul(
                            ot_sb[h][:, b * 512 : (b + 1) * 512], ot[0:64, :], invb
                        )

            if dbg:
                with tc.tile_pool(name="dbgo", bufs=2) as dbgo:
                    for h in range(HL):
                        dt3 = dbgo.tile([64, T], DT.float32, tag="dt3", name="dt3")
                        nc.vector.tensor_copy(dt3, ot_sb[h])
                        nc.gpsimd.dma_start(out=dot_d[h], in_=dt3)

            # ---- stage C: output projection ----------------------------
            with (
                tc.tile_pool(name="pY", bufs=3) as pY,
                tc.tile_pool(name="psY", bufs=2, space="PSUM") as psY,
            ):
                for ti in range(NT):
                    for n in range(2):
                        ps = psY.tile([128, 512], DT.float32, tag="yps", name="yps")
                        for h in range(HL):
                            nc.tensor.matmul(
                                ps,
                                ot_sb[h][:, ti * 128 : (ti + 1) * 128],
                                wproj_sb[h][:, n * 512 : (n + 1) * 512],
                                start=(h == 0),
                                stop=(h == HL - 1),
                            )
                        ysb = pY.tile([128, 512], DT.float32, tag="ysb", name="ysb")
                        nc.vector.tensor_copy(ysb, ps)
                        nc.gpsimd.dma_start(
                            out=y_d[ti * 128 : (ti + 1) * 128, n * 512 : (n + 1) * 512],
                            in_=ysb,
                        )
    return nc


def _make_masks():
    p = np.arange(128)[:, None]
    c = np.arange(512)[None, :]
    return np.stack([(c >= p + 128 * k) for k in range(4)]).astype(BF16)


def prep_inputs(x, scale, w_qkv, w_proj):
    """Per-core input dict list. Core c: batch c//2, head-group c%2."""
    x = np.asarray(x, np.float32)
    scale = np.asarray(scale, np.float32)
    w_qkv = np.asarray(w_qkv, np.float32)
    w_proj = np.asarray(w_proj, np.float32)
    ws = w_qkv * scale[None, :]  # fold RMSNorm scale into the weights
    masks = _make_masks()
    in_maps = []
    for c in range(N_CORES):
        b, g = c // 2, c % 2
        rows = slice(g * FL, (g + 1) * FL)
        wq = ws[0:E][rows]
        wk = ws[E : 2 * E][rows]
        wv = ws[2 * E : 3 * E][rows]
        in_maps.append(
            {
                "xt": np.ascontiguousarray(x[b].T).astype(BF16),
                "wqk": np.ascontiguousarray(np.concatenate([wq, wk], 0).T).astype(BF16),
                "wv": np.ascontiguousarray(wv.T).astype(BF16),
                "wproj": np.ascontiguousarray(w_proj[:, rows].T).astype(BF16),
                "mask": masks,
            }
        )
    return in_maps


_CACHED_NC = None


def kernel(x, scale, w_qkv, w_proj):
    global _CACHED_NC
    if _CACHED_NC is None:
        _CACHED_NC = build_program()
    in_maps = prep_inputs(x, scale, w_qkv, w_proj)
    res = run_bass_kernel_spmd(_CACHED_NC, in_maps, list(range(N_CORES)))
    out = np.zeros((B, T, E), np.float32)
    for c in range(N_CORES):
        out[c // 2] += res.results[c]["y"]
    return out


if __name__ == "__main__":
    rng = np.random.default_rng(0)
    x = rng.standard_normal((B, T, E), dtype=np.float32)
    scale = np.ones(E, np.float32)
    w_qkv = rng.standard_normal((3 * E, E), dtype=np.float32) / 32
    w_proj = rng.standard_normal((E, E), dtype=np.float32) / 32
    y = kernel(x, scale, w_qkv, w_proj)
    print("ran", y.shape, y.dtype, np.abs(y).mean())



# revision 13
# speedup vs baseline: 1.2654x; 1.2654x over previous
"""Causal self-attention (RMSNorm + QKV + causal attention + out-proj) on 8 trn2
NeuronCores.

Sharding: core c handles batch b=c//2 and head-group g=c%2 (Megatron-style TP
over the 16 heads: 8 heads per group). Each core computes a partial output
y_part[b,g] = attn_out(heads of g) @ w_proj[:, g-cols].T ; the host sums the two
TP partials per batch.

On-core pipeline (T=1024, E=1024, 8 local heads, D=64), all matmuls in bf16
with fp32 PSUM accumulation:
  1. qk^T[f,t] tiles = (Wqk x^T), RMSNorm scale r[t] folded in at eviction.
     r comes from ACT squares + DVE add tree + a bf16 ones-matmul partition
     reduce; broadcast to 128 partitions with a bf16 K=1 matmul.
  2. V[t,f] = (xn W_v^T) with xn = x^T * r, stored [128, 8, 65] with a
     ones-column per head so the attention matmul also produces softmax
     denominators.
  3. Attention per head-pair (heads 2p/2p+1 live in partitions 0-63/64-127 of
     the same qk tile): S^T tiles for both heads are computed CONCURRENTLY via
     PE row-tiling (tile_position rows 0/64, K=64 each). P = exp(S/8) with a
     triangular 128x128 mask on the diagonal block only; causal trimming skips
     fully-masked columns. O^T (+denominator row) = V'.T @ P; normalization by
     the reciprocal denominator broadcast across partitions on GpSimd.
  4. y[t,:] partial: row-packed K=64 matmul pairs sum head contributions into
     two PSUM banks (even/odd heads), DVE-added and DMA'd out in fp32.
     proj for the first 512 tokens is interleaved into the second attention
     half to keep the PE dense.
"""

import sys

sys.path.insert(0, "/opt/trn_rl_repo")

import numpy as np
import ml_dtypes

import concourse.bass as bass
import concourse.tile as tile
from concourse import mybir
from concourse.bass_utils import run_bass_kernel_spmd

BF16 = ml_dtypes.bfloat16

B, T, E, H = 4, 1024, 1024, 16
D = E // H  # 64
HL = 8  # heads per core (TP degree 2)
FL = HL * D  # 512 local head cols
EPS = 1e-5
N_CORES = 8
NT = T // 128  # 8 tiles of 128

AF = mybir.ActivationFunctionType
DT = mybir.dt


# ---------------------------------------------------------------------------
# Walrus in this toolchain rejects instructions whose tail Drain carries more
# than one semaphore wait; split the TileContext exit drain into chained
# single-wait drains.
def _patched_drain_and_barrier(self, tick_clock, wait_clock):
    nc = self.nc
    drain_inst = nc.sync.drain()
    wait_clock.add_sem_waits(
        drain_inst.ins, tile.ScopedClock({None: tick_clock.global_clock})
    )
    mi = drain_inst.ins
    si = mi.sync_info
    if si is not None and len(si.on_wait) > 1:
        waits = list(si.on_wait)
        mi.sync_info = mybir.SyncInfo(on_wait=waits[:1], on_update=list(si.on_update))
        for w in waits[1:]:
            extra = nc.sync.drain().ins
            extra.sync_info = mybir.SyncInfo(on_wait=[w], on_update=[])
    nc.all_engine_barrier()
    assert self.sems is not None
    popped = nc._tile_sem_poison_stack.pop()
    assert popped is self._sem_poison
    nc.clear_and_free_semaphores(list(self.sems.allocated().values()))
    nc.all_engine_barrier()


tile.TileContext._drain_and_barrier = _patched_drain_and_barrier

# The same 1-wait-per-instruction walrus limit applies to every engine's
# instructions. Legalize at BIR-serialization time: hoist all but the last
# wait of a multi-wait instruction onto fresh single-wait Drains inserted
# just before it on the same engine.
_orig_to_json_bytes = bass.Bass.to_json_bytes


def _legalized_to_json_bytes(self):
    import orjson

    j = orjson.loads(_orig_to_json_bytes(self))
    ctr = 0
    for fn in j["functions"]:
        for bb in fn["blocks"]:
            new_insts = []
            for ins in bb["instructions"]:
                si = ins.get("sync_info")
                waits = si.get("on_wait") if si else None
                if waits and len(waits) > 1:
                    for w in waits[:-1]:
                        ctr += 1
                        new_insts.append(
                            {
                                "debug": ins.get("debug"),
                                "engine": ins["engine"],
                                "ins": [],
                                "outs": [],
                                "name": f"I-wf{ctr}",
                                "opcode": "EventSemaphore",
                                "sync_info": {"on_update": [], "on_wait": [w]},
                            }
                        )
                    si["on_wait"] = [waits[-1]]
                new_insts.append(ins)
            bb["instructions"] = new_insts
    return orjson.dumps(j)


bass.Bass.to_json_bytes = _legalized_to_json_bytes


def build_program():
    nc = bass.Bass("TRN2", target_bir_lowering=False, debug=False)

    xt_d = nc.declare_dram_parameter("xt", [E, T], DT.bfloat16, isOutput=False)
    wqk_d = nc.declare_dram_parameter("wqk", [E, 2 * FL], DT.bfloat16, isOutput=False)
    wv_d = nc.declare_dram_parameter("wv", [E, FL], DT.bfloat16, isOutput=False)
    wproj_d = nc.declare_dram_parameter("wproj", [FL, E], DT.bfloat16, isOutput=False)
    mask_d = nc.declare_dram_parameter("mask", [128, 128], DT.bfloat16, isOutput=False)
    y_d = nc.declare_dram_parameter("y", [T, E], DT.float32, isOutput=True)
    import os

    dbg = os.environ.get("KDBG") == "1"
    if dbg:
        dqk_d = nc.declare_dram_parameter("dqk", [8, 128, T], DT.float32, isOutput=True)
        dvp_d = nc.declare_dram_parameter(
            "dvp", [8, 128, HL * 65], DT.float32, isOutput=True
        )
        dot_d = nc.declare_dram_parameter("dot", [4, 128, T], DT.float32, isOutput=True)

    with tile.TileContext(nc) as tc:
        with (
            tc.tile_pool(name="persist", bufs=1) as persist,
        ):
            # ---- persistent SBUF tensors -------------------------------
            qk_sb = [
                persist.tile([128, T], DT.bfloat16, tag=f"qk{i}", name=f"qk{i}")
                for i in range(8)
            ]
            vp_sb = [
                persist.tile([128, HL, 65], DT.bfloat16, tag=f"vp{i}", name=f"vp{i}")
                for i in range(NT)
            ]
            # ot_pair[p]: head 2p on partitions 0-63, head 2p+1 on 64-127
            ot_pair = [
                persist.tile([128, T], DT.bfloat16, tag=f"ot{p}", name=f"ot{p}")
                for p in range(4)
            ]
            wproj_sb = [
                persist.tile([128, E], DT.bfloat16, tag=f"wp{p}", name=f"wp{p}")
                for p in range(4)
            ]
            mask_sb = persist.tile([128, 128], DT.bfloat16, tag="mk", name="mk")
            r_bcast = persist.tile([128, T], DT.bfloat16, tag="r_bcast", name="r_bcast")
            ones_col = persist.tile([128, 1], DT.bfloat16, tag="ones_col", name="ones_col")
            ones_row = persist.tile([1, 128], DT.bfloat16, tag="ones_row", name="ones_row")
            r_sb = persist.tile([1, T], DT.bfloat16, tag="r_sb", name="r_sb")
            s_sb = persist.tile([1, T], DT.float32, tag="s_sb", name="s_sb")
            eps_t = persist.tile([1, 1], DT.float32, tag="eps_t", name="eps_t")
            nc.vector.memset(eps_t, float(EPS))
            nc.vector.memset(ones_col, 1.0)
            nc.vector.memset(ones_row, 1.0)
            for i in range(NT):
                nc.vector.memset(vp_sb[i][:, :, 64:65], 1.0)

            # ---- DMA: x/wqk interleaved on sync queue; rest on gpsimd --
            with (
                tc.tile_pool(name="xw", bufs=1) as xw,
                tc.tile_pool(name="psA", bufs=6, space="PSUM") as psA,
                tc.tile_pool(name="psS", bufs=2, space="PSUM") as psS,
            ):
                xt_sb = [
                    xw.tile([128, T], DT.bfloat16, tag=f"xt{i}", name=f"xt{i}")
                    for i in range(8)
                ]
                wqk_sb = [
                    xw.tile([128, 2 * FL], DT.bfloat16, tag=f"wqk{i}", name=f"wqk{i}")
                    for i in range(8)
                ]
                wv_sb = [
                    xw.tile([128, FL], DT.bfloat16, tag=f"wv{i}", name=f"wv{i}")
                    for i in range(8)
                ]
                for i in range(8):
                    nc.sync.dma_start(out=xt_sb[i], in_=xt_d[i * 128 : (i + 1) * 128, :])
                    nc.sync.dma_start(out=wqk_sb[i], in_=wqk_d[i * 128 : (i + 1) * 128, :])
                for i in range(8):
                    nc.gpsimd.dma_start(out=wv_sb[i], in_=wv_d[i * 128 : (i + 1) * 128, :])
                for p in range(4):
                    nc.gpsimd.dma_start(
                        out=wproj_sb[p], in_=wproj_d[p * 128 : (p + 1) * 128, :]
                    )
                nc.gpsimd.dma_start(out=mask_sb, in_=mask_d[0:128, :])

                # ---- norm stats (ACT squares + DVE tree + bf16 ones-MM) --
                sq = [
                    xw.tile([128, T], DT.float32, tag=f"sq{i}", name=f"sq{i}")
                    for i in range(8)
                ]
                acc_bf = xw.tile([128, T], DT.bfloat16, tag="acc_bf", name="acc_bf")
                for i in range(8):
                    nc.scalar.square(sq[i], xt_sb[i])
                for step in (1, 2):
                    for i in range(0, 8, 2 * step):
                        nc.vector.tensor_add(sq[i], sq[i], sq[i + step])
                nc.vector.tensor_add(acc_bf, sq[0], sq[4])

                # ---- stage A matmul stream -------------------------------
                def emit_qk_mms(fi):
                    ps = [
                        psA.tile([128, 512], DT.float32, tag="qkps", name="qkps")
                        for _ in range(2)
                    ]
                    for ei in range(8):
                        for n in range(2):
                            nc.tensor.matmul(
                                ps[n],
                                wqk_sb[ei][:, fi * 128 : (fi + 1) * 128],
                                xt_sb[ei][:, n * 512 : (n + 1) * 512],
                                start=(ei == 0),
                                stop=(ei == 7),
                            )
                    return ps

                def emit_qk_evict(fi, ps):
                    # NOTE: must be emitted after the r_bcast writes (program
                    # order defines dataflow for the tile framework)
                    for n in range(2):
                        nc.vector.tensor_mul(
                            qk_sb[fi][:, n * 512 : (n + 1) * 512],
                            ps[n],
                            r_bcast[:, n * 512 : (n + 1) * 512],
                        )

                def emit_qk(fi):
                    emit_qk_evict(fi, emit_qk_mms(fi))

                ps0 = emit_qk_mms(0)
                ps4 = emit_qk_mms(4)

                # sumsq + sqrt + recip + broadcast
                sumsq = [
                    psS.tile([1, 512], DT.float32, tag="ss", name="ss") for _ in range(2)
                ]
                for n in range(2):
                    nc.tensor.matmul(
                        sumsq[n],
                        ones_col,
                        acc_bf[:, n * 512 : (n + 1) * 512],
                        start=True,
                        stop=True,
                    )
                    nc.scalar.activation(
                        s_sb[:, n * 512 : (n + 1) * 512],
                        sumsq[n],
                        AF.Sqrt,
                        bias=eps_t,
                        scale=1.0 / E,
                    )
                with nc.allow_low_precision(reason="r is a ~1.0 per-token scale"):
                    nc.vector.reciprocal(r_sb, s_sb)
                for n in range(2):
                    rbp = psA.tile([128, 512], DT.float32, tag="qkps", name="rbp")
                    nc.tensor.matmul(
                        rbp,
                        ones_row,
                        r_sb[:, n * 512 : (n + 1) * 512],
                        start=True,
                        stop=True,
                    )
                    nc.vector.tensor_copy(r_bcast[:, n * 512 : (n + 1) * 512], rbp)
                emit_qk_evict(0, ps0)
                emit_qk_evict(4, ps4)

                # normalized x for the V path
                xn_sb = [
                    xw.tile([128, T], DT.bfloat16, tag=f"xn{i}", name=f"xn{i}")
                    for i in range(8)
                ]
                for i in range(8):
                    nc.vector.tensor_mul(xn_sb[i], xt_sb[i], r_bcast)

                for fi in (1, 5, 2, 6, 3, 7):
                    emit_qk(fi)

                # V natural: [t=128, 8, 64] = sum_e xn_e(cols ti).T @ wv_e
                for ti in range(NT):
                    ps = psA.tile([128, 8, 64], DT.float32, tag="qkps", name="vps")
                    for ei in range(8):
                        nc.tensor.matmul(
                            ps,
                            xn_sb[ei][:, ti * 128 : (ti + 1) * 128],
                            wv_sb[ei],
                            start=(ei == 0),
                            stop=(ei == 7),
                        )
                    nc.scalar.activation(vp_sb[ti][:, :, 0:64], ps, AF.Copy)

            if dbg:
                with tc.tile_pool(name="dbgp", bufs=2) as dbgp:
                    for i in range(8):
                        dt1 = dbgp.tile([128, T], DT.float32, tag="dt1", name="dt1")
                        nc.vector.tensor_copy(dt1, qk_sb[i])
                        nc.gpsimd.dma_start(out=dqk_d[i], in_=dt1)
                        dt2 = dbgp.tile([128, HL, 65], DT.float32, tag="dt2", name="dt2")
                        nc.vector.tensor_copy(dt2, vp_sb[i])
                        nc.gpsimd.dma_start(out=dvp_d[i], in_=dt2)

            # ---- stage B+C: attention + interleaved projection ----------
            with (
                tc.tile_pool(name="pP", bufs=4) as pP,
                tc.tile_pool(name="pInv", bufs=3) as pInv,
                tc.tile_pool(name="pY", bufs=3) as pY,
                tc.tile_pool(name="psST", bufs=2, space="PSUM") as psST,
                tc.tile_pool(name="psOT", bufs=1, space="PSUM") as psOT,
                tc.tile_pool(name="psY", bufs=1, space="PSUM") as psY,
            ):

                def emit_attn(p, b):
                    """Attention for head pair p (heads 2p, 2p+1), q-chunk b."""
                    nj = 4 * b + 4
                    ot_e = psOT.tile([65, 512], DT.float32, tag="oe", name="oe")
                    ot_o = psOT.tile([65, 512], DT.float32, tag="oo", name="oo")
                    kt = qk_sb[4 + p]
                    qt = qk_sb[p]
                    for j in range(nj):
                        qoff = max(0, j * 128 - b * 512)
                        qs = b * 512 + qoff  # absolute first q column
                        st_e = psST.tile([128, 512], DT.float32, tag="se", name="se")
                        st_o = psST.tile([128, 512], DT.float32, tag="so", name="so")
                        nc.tensor.matmul(
                            st_e[:, qoff:512],
                            kt[0:64, j * 128 : (j + 1) * 128],
                            qt[0:64, qs : (b + 1) * 512],
                            start=True,
                            stop=True,
                        )
                        nc.tensor.matmul(
                            st_o[:, qoff:512],
                            kt[64:128, j * 128 : (j + 1) * 128],
                            qt[64:128, qs : (b + 1) * 512],
                            start=True,
                            stop=True,
                        )
                        p_e = pP.tile([128, 512], DT.bfloat16, tag="pe", name="p_e")
                        p_o = pP.tile([128, 512], DT.bfloat16, tag="po", name="p_o")
                        diag = qs == j * 128  # block straddles the diagonal
                        for st, p_t in ((st_e, p_e), (st_o, p_o)):
                            if diag:
                                praw = pP.tile(
                                    [128, 128], DT.bfloat16, tag="praw", name="praw"
                                )
                                nc.scalar.activation(
                                    praw, st[:, qoff : qoff + 128], AF.Exp, scale=0.125
                                )
                                nc.vector.tensor_mul(
                                    p_t[:, qoff : qoff + 128], praw, mask_sb
                                )
                                if qoff + 128 < 512:
                                    nc.scalar.activation(
                                        p_t[:, qoff + 128 : 512],
                                        st[:, qoff + 128 : 512],
                                        AF.Exp,
                                        scale=0.125,
                                    )
                            else:
                                nc.scalar.activation(
                                    p_t[:, qoff:512], st[:, qoff:512], AF.Exp, scale=0.125
                                )
                        nc.tensor.matmul(
                            ot_e[:, qoff:512],
                            vp_sb[j][:, 2 * p : 2 * p + 1, :],
                            p_e[:, qoff:512],
                            start=(j == 0),
                            stop=(j == nj - 1),
                        )
                        nc.tensor.matmul(
                            ot_o[:, qoff:512],
                            vp_sb[j][:, 2 * p + 1 : 2 * p + 2, :],
                            p_o[:, qoff:512],
                            start=(j == 0),
                            stop=(j == nj - 1),
                        )
                    # normalize by softmax denominator (row 64): K=1 ones-matmul
                    # broadcasts each head's reciprocal row across 64 partitions
                    for par, ot_ps in ((0, ot_e), (1, ot_o)):
                        inv = pInv.tile([1, 512], DT.bfloat16, tag=f"iv{par}", name="inv")
                        with nc.allow_low_precision(reason="softmax denom recip"):
                            nc.vector.reciprocal(inv, ot_ps[64:65, :])
                        ibp = psST.tile([128, 512], DT.float32, tag="se", name="ibp")
                        nc.tensor.matmul(
                            ibp[0:64, :], ones_row[0:1, 0:64], inv, start=True, stop=True
                        )
                        invb = pInv.tile(
                            [64, 512], DT.float32, tag=f"ib{par}", name="invb"
                        )
                        nc.scalar.activation(invb, ibp[0:64, :], AF.Copy)
                        nc.vector.tensor_mul(
                            ot_pair[p][
                                64 * par : 64 * par + 64, b * 512 : (b + 1) * 512
                            ],
                            ot_ps[0:64, :],
                            invb,
                        )

                def emit_proj(ti, n):
                    """y[t-tile ti, n-chunk] = sum over head pairs (row-packed)."""
                    ye = psY.tile([128, 512], DT.float32, tag="ye", name="ye")
                    yo = psY.tile([128, 512], DT.float32, tag="yo", name="yo")
                    for p in range(4):
                        nc.tensor.matmul(
                            ye,
                            ot_pair[p][0:64, ti * 128 : (ti + 1) * 128],
                            wproj_sb[p][0:64, n * 512 : (n + 1) * 512],
                            start=(p == 0),
                            stop=(p == 3),
                        )
                        nc.tensor.matmul(
                            yo,
                            ot_pair[p][64:128, ti * 128 : (ti + 1) * 128],
                            wproj_sb[p][64:128, n * 512 : (n + 1) * 512],
                            start=(p == 0),
                            stop=(p == 3),
                        )
                    # DVE cannot read two PSUM inputs; stage even half via ACT
                    yeb = pY.tile([128, 512], DT.float32, tag="yeb", name="yeb")
                    nc.scalar.activation(yeb, ye, AF.Copy)
                    ysb = pY.tile([128, 512], DT.float32, tag="ysb", name="ysb")
                    nc.vector.tensor_add(ysb, yeb, yo)
                    nc.gpsimd.dma_start(
                        out=y_d[ti * 128 : (ti + 1) * 128, n * 512 : (n + 1) * 512],
                        in_=ysb,
                    )

                for p in range(4):
                    emit_attn(p, 0)
                for p in range(4):
                    emit_attn(p, 1)
                    # interleave first-half projection into second-half attention
                    emit_proj(p, 0)
                    emit_proj(p, 1)
                for ti in range(4, 8):
                    for n in range(2):
                        emit_proj(ti, n)

            if dbg:
                with tc.tile_pool(name="dbgo", bufs=2) as dbgo:
                    for p in range(4):
                        dt3 = dbgo.tile([128, T], DT.float32, tag="dt3", name="dt3")
                        nc.vector.tensor_copy(dt3, ot_pair[p])
                        nc.gpsimd.dma_start(out=dot_d[p], in_=dt3)
    return nc


def _make_mask():
    p = np.arange(128)[:, None]
    c = np.arange(128)[None, :]
    return (c >= p).astype(BF16)


def prep_inputs(x, scale, w_qkv, w_proj):
    """Per-core input dict list. Core c: batch c//2, head-group c%2."""
    x = np.asarray(x, np.float32)
    scale = np.asarray(scale, np.float32)
    w_qkv = np.asarray(w_qkv, np.float32)
    w_proj = np.asarray(w_proj, np.float32)
    ws = w_qkv * scale[None, :]  # fold RMSNorm scale into the weights
    mask = _make_mask()
    in_maps = []
    for c in range(N_CORES):
        b, g = c // 2, c % 2
        rows = slice(g * FL, (g + 1) * FL)
        wq = ws[0:E][rows]
        wk = ws[E : 2 * E][rows]
        wv = ws[2 * E : 3 * E][rows]
        in_maps.append(
            {
                "xt": np.ascontiguousarray(x[b].T).astype(BF16),
                "wqk": np.ascontiguousarray(np.concatenate([wq, wk], 0).T).astype(BF16),
                "wv": np.ascontiguousarray(wv.T).astype(BF16),
                "wproj": np.ascontiguousarray(w_proj[:, rows].T).astype(BF16),
                "mask": mask,
            }
        )
    return in_maps


_CACHED_NC = None


def kernel(x, scale, w_qkv, w_proj):
    global _CACHED_NC
    if _CACHED_NC is None:
        _CACHED_NC = build_program()
    in_maps = prep_inputs(x, scale, w_qkv, w_proj)
    res = run_bass_kernel_spmd(_CACHED_NC, in_maps, list(range(N_CORES)))
    out = np.zeros((B, T, E), np.float32)
    for c in range(N_CORES):
        out[c // 2] += res.results[c]["y"]
    return out


if __name__ == "__main__":
    rng = np.random.default_rng(0)
    x = rng.standard_normal((B, T, E), dtype=np.float32)
    scale = np.ones(E, np.float32)
    w_qkv = rng.standard_normal((3 * E, E), dtype=np.float32) / 32
    w_proj = rng.standard_normal((E, E), dtype=np.float32) / 32
    y = kernel(x, scale, w_qkv, w_proj)
    print("ran", y.shape, y.dtype, np.abs(y).mean())


# revision 16
# speedup vs baseline: 1.4502x; 1.1460x over previous
"""Causal self-attention (RMSNorm + QKV + causal attention + out-proj) on 8 trn2
NeuronCores.

Sharding: core c handles batch b=c//2 and head-group g=c%2 (Megatron-style TP
over the 16 heads: 8 heads per group). Each core computes a partial output
y_part[b,g] = attn_out(heads of g) @ w_proj[:, g-cols].T ; the host sums the two
TP partials per batch.

On-core pipeline (T=1024, E=1024, 8 local heads, D=64), all matmuls in bf16
with fp32 PSUM accumulation:
  1. qk^T[f,t] tiles = (Wqk x^T), RMSNorm scale r[t] folded in at eviction.
     r comes from ACT squares + DVE add tree + a bf16 ones-matmul partition
     reduce; broadcast to 128 partitions with a bf16 K=1 matmul.
  2. V[t,f] = (xn W_v^T) with xn = x^T * r, stored [128, 8, 65] with a
     ones-column per head so the attention matmul also produces softmax
     denominators.
  3. Attention per head-pair (heads 2p/2p+1 live in partitions 0-63/64-127 of
     the same qk tile): S^T tiles for both heads are computed CONCURRENTLY via
     PE row-tiling (tile_position rows 0/64, K=64 each). P = exp(S/8) with a
     triangular 128x128 mask on the diagonal block only; causal trimming skips
     fully-masked columns. O^T (+denominator row) = V'.T @ P; normalization by
     the reciprocal denominator broadcast across partitions on GpSimd.
  4. y[t,:] partial: row-packed K=64 matmul pairs sum head contributions into
     two PSUM banks (even/odd heads), DVE-added and DMA'd out in fp32.
     proj for the first 512 tokens is interleaved into the second attention
     half to keep the PE dense.
"""

import sys

sys.path.insert(0, "/opt/trn_rl_repo")

import numpy as np
import ml_dtypes

import concourse.bass as bass
import concourse.tile as tile
from concourse import mybir
from concourse.bass_utils import run_bass_kernel_spmd

BF16 = ml_dtypes.bfloat16

B, T, E, H = 4, 1024, 1024, 16
D = E // H  # 64
HL = 8  # heads per core (TP degree 2)
FL = HL * D  # 512 local head cols
EPS = 1e-5
N_CORES = 8
NT = T // 128  # 8 tiles of 128

AF = mybir.ActivationFunctionType
DT = mybir.dt


# ---------------------------------------------------------------------------
# Walrus in this toolchain rejects instructions whose tail Drain carries more
# than one semaphore wait; split the TileContext exit drain into chained
# single-wait drains.
def _patched_drain_and_barrier(self, tick_clock, wait_clock):
    nc = self.nc
    drain_inst = nc.sync.drain()
    wait_clock.add_sem_waits(
        drain_inst.ins, tile.ScopedClock({None: tick_clock.global_clock})
    )
    mi = drain_inst.ins
    si = mi.sync_info
    if si is not None and len(si.on_wait) > 1:
        waits = list(si.on_wait)
        mi.sync_info = mybir.SyncInfo(on_wait=waits[:1], on_update=list(si.on_update))
        for w in waits[1:]:
            extra = nc.sync.drain().ins
            extra.sync_info = mybir.SyncInfo(on_wait=[w], on_update=[])
    nc.all_engine_barrier()
    assert self.sems is not None
    popped = nc._tile_sem_poison_stack.pop()
    assert popped is self._sem_poison
    nc.clear_and_free_semaphores(list(self.sems.allocated().values()))
    nc.all_engine_barrier()


tile.TileContext._drain_and_barrier = _patched_drain_and_barrier

# The same 1-wait-per-instruction walrus limit applies to every engine's
# instructions. Legalize at BIR-serialization time: hoist all but the last
# wait of a multi-wait instruction onto fresh single-wait Drains inserted
# just before it on the same engine.
_orig_to_json_bytes = bass.Bass.to_json_bytes


def _legalized_to_json_bytes(self):
    import orjson

    j = orjson.loads(_orig_to_json_bytes(self))
    ctr = 0
    for fn in j["functions"]:
        for bb in fn["blocks"]:
            new_insts = []
            for ins in bb["instructions"]:
                si = ins.get("sync_info")
                waits = si.get("on_wait") if si else None
                if waits and len(waits) > 1:
                    for w in waits[:-1]:
                        ctr += 1
                        new_insts.append(
                            {
                                "debug": ins.get("debug"),
                                "engine": ins["engine"],
                                "ins": [],
                                "outs": [],
                                "name": f"I-wf{ctr}",
                                "opcode": "EventSemaphore",
                                "sync_info": {"on_update": [], "on_wait": [w]},
                            }
                        )
                    si["on_wait"] = [waits[-1]]
                new_insts.append(ins)
            bb["instructions"] = new_insts
    return orjson.dumps(j)


bass.Bass.to_json_bytes = _legalized_to_json_bytes

# Enable walrus LDWEIGHTS dedup (consecutive matmuls sharing a stationary
# operand skip the reload). bass_utils hardcodes it off; rewrite the argv.
import os as _os
# NOTE: walrus rejects ldw-opt with this kernel's Ldweights mix
# ("InstLdweights is not compatible with LDW optimization") — keep off.
if _os.environ.get("KLDW", "0") == "1":
    from concourse import bass_utils as _bu

    _orig_run_command = _bu.run_command

    def _ldw_run_command(cmd, *a, **kw):
        cmd = [
            c.replace("--enable-ldw-opt=false", "--enable-ldw-opt=true")
            if isinstance(c, str)
            else c
            for c in cmd
        ]
        return _orig_run_command(cmd, *a, **kw)

    _bu.run_command = _ldw_run_command


def build_program():
    nc = bass.Bass("TRN2", target_bir_lowering=False, debug=False)

    xt_d = nc.declare_dram_parameter("xt", [E, T], DT.bfloat16, isOutput=False)
    wqk_d = nc.declare_dram_parameter("wqk", [E, 2 * FL], DT.bfloat16, isOutput=False)
    wv_d = nc.declare_dram_parameter("wv", [E, FL], DT.bfloat16, isOutput=False)
    wproj_d = nc.declare_dram_parameter("wproj", [FL, E], DT.bfloat16, isOutput=False)
    mask_d = nc.declare_dram_parameter("mask", [128, 128], DT.bfloat16, isOutput=False)
    y_d = nc.declare_dram_parameter("y", [T, E], DT.float32, isOutput=True)
    import os

    dbg = os.environ.get("KDBG") == "1"
    if dbg:
        dqk_d = nc.declare_dram_parameter("dqk", [8, 128, T], DT.float32, isOutput=True)
        dvp_d = nc.declare_dram_parameter(
            "dvp", [8, 128, HL * 65], DT.float32, isOutput=True
        )
        dot_d = nc.declare_dram_parameter("dot", [4, 128, T], DT.float32, isOutput=True)

    with tile.TileContext(nc) as tc:
        with (
            tc.tile_pool(name="persist", bufs=1) as persist,
        ):
            # ---- persistent SBUF tensors -------------------------------
            qk_sb = [
                persist.tile([128, T], DT.bfloat16, tag=f"qk{i}", name=f"qk{i}")
                for i in range(8)
            ]
            vp_sb = [
                persist.tile([128, HL, 65], DT.bfloat16, tag=f"vp{i}", name=f"vp{i}")
                for i in range(NT)
            ]
            # ot_pair[p]: head 2p on partitions 0-63, head 2p+1 on 64-127
            ot_pair = [
                persist.tile([128, T], DT.bfloat16, tag=f"ot{p}", name=f"ot{p}")
                for p in range(4)
            ]
            wproj_sb = [
                persist.tile([128, E], DT.bfloat16, tag=f"wp{p}", name=f"wp{p}")
                for p in range(4)
            ]
            mask_sb = persist.tile([128, 128], DT.bfloat16, tag="mk", name="mk")
            r_bcast = persist.tile([128, T], DT.bfloat16, tag="r_bcast", name="r_bcast")
            ones_col = persist.tile([128, 1], DT.bfloat16, tag="ones_col", name="ones_col")
            ones_row = persist.tile([1, 128], DT.bfloat16, tag="ones_row", name="ones_row")
            r_sb = persist.tile([1, T], DT.bfloat16, tag="r_sb", name="r_sb")
            lnm_sb = persist.tile([1, T], DT.float32, tag="lnm_sb", name="lnm_sb")
            eps_t = persist.tile([1, 1], DT.float32, tag="eps_t", name="eps_t")
            nc.vector.memset(eps_t, float(EPS))
            nc.vector.memset(ones_col, 1.0)
            nc.vector.memset(ones_row, 1.0)
            for i in range(NT):
                nc.vector.memset(vp_sb[i][:, :, 64:65], 1.0)

            # ---- DMA: x/wqk interleaved on sync queue; rest on gpsimd --
            with (
                tc.tile_pool(name="xw", bufs=1) as xw,
                tc.tile_pool(name="psA", bufs=6, space="PSUM") as psA,
                tc.tile_pool(name="psS", bufs=2, space="PSUM") as psS,
            ):
                xt_sb = [
                    xw.tile([128, T], DT.bfloat16, tag=f"xt{i}", name=f"xt{i}")
                    for i in range(8)
                ]
                wqk_sb = [
                    xw.tile([128, 2 * FL], DT.bfloat16, tag=f"wqk{i}", name=f"wqk{i}")
                    for i in range(8)
                ]
                wv_sb = [
                    xw.tile([128, FL], DT.bfloat16, tag=f"wv{i}", name=f"wv{i}")
                    for i in range(8)
                ]
                # spread input loads over four queues so the PE is fed early
                qs_ = [nc.sync, nc.sync, nc.gpsimd, nc.scalar]
                for i in range(8):
                    qs_[i % 2].dma_start(out=xt_sb[i], in_=xt_d[i * 128 : (i + 1) * 128, :])
                    qs_[2 + i % 2].dma_start(
                        out=wqk_sb[i], in_=wqk_d[i * 128 : (i + 1) * 128, :]
                    )
                for i in range(8):
                    qs_[i % 2].dma_start(out=wv_sb[i], in_=wv_d[i * 128 : (i + 1) * 128, :])
                for p in range(4):
                    nc.gpsimd.dma_start(
                        out=wproj_sb[p], in_=wproj_d[p * 128 : (p + 1) * 128, :]
                    )
                nc.gpsimd.dma_start(out=mask_sb, in_=mask_d[0:128, :])

                # ---- norm stats (ACT squares + DVE tree + bf16 ones-MM) --
                sq = [
                    xw.tile([128, T], DT.float32, tag=f"sq{i}", name=f"sq{i}")
                    for i in range(8)
                ]
                acc_bf = xw.tile([128, T], DT.bfloat16, tag="acc_bf", name="acc_bf")
                for i in range(8):
                    nc.scalar.square(sq[i], xt_sb[i])
                for step in (1, 2):
                    for i in range(0, 8, 2 * step):
                        nc.vector.tensor_add(sq[i], sq[i], sq[i + step])
                nc.vector.tensor_add(acc_bf, sq[0], sq[4])

                # ---- stage A matmul stream -------------------------------
                def emit_qk_mms(fi):
                    ps = [
                        psA.tile([128, 512], DT.float32, tag="qkps", name="qkps")
                        for _ in range(2)
                    ]
                    for ei in range(8):
                        for n in range(2):
                            nc.tensor.matmul(
                                ps[n],
                                wqk_sb[ei][:, fi * 128 : (fi + 1) * 128],
                                xt_sb[ei][:, n * 512 : (n + 1) * 512],
                                start=(ei == 0),
                                stop=(ei == 7),
                            )
                    return ps

                def emit_qk_evict(fi, ps):
                    # NOTE: must be emitted after the r_bcast writes (program
                    # order defines dataflow for the tile framework)
                    for n in range(2):
                        nc.vector.tensor_mul(
                            qk_sb[fi][:, n * 512 : (n + 1) * 512],
                            ps[n],
                            r_bcast[:, n * 512 : (n + 1) * 512],
                        )

                def emit_qk(fi):
                    emit_qk_evict(fi, emit_qk_mms(fi))

                ps0 = emit_qk_mms(0)
                ps4 = emit_qk_mms(4)

                # sumsq + sqrt + recip + broadcast
                sumsq = [
                    psS.tile([1, 512], DT.float32, tag="ss", name="ss") for _ in range(2)
                ]
                for n in range(2):
                    nc.tensor.matmul(
                        sumsq[n],
                        ones_col,
                        acc_bf[:, n * 512 : (n + 1) * 512],
                        start=True,
                        stop=True,
                    )
                    nc.scalar.activation(
                        lnm_sb[:, n * 512 : (n + 1) * 512],
                        sumsq[n],
                        AF.Ln,
                        bias=eps_t,
                        scale=1.0 / E,
                    )
                # r = exp(-0.5 * ln(mean_sq + eps)) = rsqrt(mean_sq + eps)
                nc.scalar.activation(r_sb, lnm_sb, AF.Exp, scale=-0.5)
                for n in range(2):
                    rbp = psA.tile([128, 512], DT.float32, tag="qkps", name="rbp")
                    nc.tensor.matmul(
                        rbp,
                        ones_row,
                        r_sb[:, n * 512 : (n + 1) * 512],
                        start=True,
                        stop=True,
                    )
                    nc.vector.tensor_copy(r_bcast[:, n * 512 : (n + 1) * 512], rbp)
                emit_qk_evict(0, ps0)
                emit_qk_evict(4, ps4)

                # normalized x for the V path
                xn_sb = [
                    xw.tile([128, T], DT.bfloat16, tag=f"xn{i}", name=f"xn{i}")
                    for i in range(8)
                ]
                for i in range(8):
                    nc.vector.tensor_mul(xn_sb[i], xt_sb[i], r_bcast)

                for fi in (1, 5, 2, 6, 3, 7):
                    emit_qk(fi)

                # V natural: [t=128, 8, 64] = sum_e xn_e(cols ti).T @ wv_e
                for ti in range(NT):
                    ps = psA.tile([128, 8, 64], DT.float32, tag="qkps", name="vps")
                    for ei in range(8):
                        nc.tensor.matmul(
                            ps,
                            xn_sb[ei][:, ti * 128 : (ti + 1) * 128],
                            wv_sb[ei],
                            start=(ei == 0),
                            stop=(ei == 7),
                        )
                    nc.scalar.activation(vp_sb[ti][:, :, 0:64], ps, AF.Copy)

            if dbg:
                with tc.tile_pool(name="dbgp", bufs=2) as dbgp:
                    for i in range(8):
                        dt1 = dbgp.tile([128, T], DT.float32, tag="dt1", name="dt1")
                        nc.vector.tensor_copy(dt1, qk_sb[i])
                        nc.gpsimd.dma_start(out=dqk_d[i], in_=dt1)
                        dt2 = dbgp.tile([128, HL, 65], DT.float32, tag="dt2", name="dt2")
                        nc.vector.tensor_copy(dt2, vp_sb[i])
                        nc.gpsimd.dma_start(out=dvp_d[i], in_=dt2)

            # ---- stage B+C: attention + interleaved projection ----------
            with (
                tc.tile_pool(name="pP", bufs=4) as pP,
                tc.tile_pool(name="pInv", bufs=3) as pInv,
                tc.tile_pool(name="pY", bufs=3) as pY,
                tc.tile_pool(name="psST", bufs=2, space="PSUM") as psST,
                tc.tile_pool(name="psOT", bufs=1, space="PSUM") as psOT,
                tc.tile_pool(name="psY", bufs=1, space="PSUM") as psY,
            ):

                def emit_attn(p, b, norm_cb=None):
                    """Attention for head pair p (heads 2p, 2p+1), q-chunk b.
                    Returns a closure that emits the denominator-normalize ops;
                    the caller emits it inside the NEXT block (between its first
                    S pair and its first PV) so the PE never stalls on it."""
                    nj = 4 * b + 4
                    ot_e = psOT.tile([65, 512], DT.float32, tag="oe", name="oe")
                    ot_o = psOT.tile([65, 512], DT.float32, tag="oo", name="oo")
                    kt = qk_sb[4 + p]
                    qt = qk_sb[p]
                    for j in range(nj):
                        qoff = max(0, j * 128 - b * 512)
                        qs = b * 512 + qoff  # absolute first q column
                        st_e = psST.tile([128, 512], DT.float32, tag="se", name="se")
                        st_o = psST.tile([128, 512], DT.float32, tag="so", name="so")
                        nc.tensor.matmul(
                            st_e[:, qoff:512],
                            kt[0:64, j * 128 : (j + 1) * 128],
                            qt[0:64, qs : (b + 1) * 512],
                            start=True,
                            stop=True,
                        )
                        nc.tensor.matmul(
                            st_o[:, qoff:512],
                            kt[64:128, j * 128 : (j + 1) * 128],
                            qt[64:128, qs : (b + 1) * 512],
                            start=True,
                            stop=True,
                        )
                        if j == 0 and norm_cb is not None:
                            norm_cb()
                        p_e = pP.tile([128, 512], DT.bfloat16, tag="pe", name="p_e")
                        p_o = pP.tile([128, 512], DT.bfloat16, tag="po", name="p_o")
                        diag = qs == j * 128  # block straddles the diagonal
                        for st, p_t in ((st_e, p_e), (st_o, p_o)):
                            if diag:  # additive -30000 mask, then exp -> 0
                                nc.vector.tensor_add(
                                    st[:, qoff : qoff + 128],
                                    st[:, qoff : qoff + 128],
                                    mask_sb,
                                )
                            nc.scalar.activation(
                                p_t[:, qoff:512], st[:, qoff:512], AF.Exp, scale=0.125
                            )
                        nc.tensor.matmul(
                            ot_e[:, qoff:512],
                            vp_sb[j][:, 2 * p : 2 * p + 1, :],
                            p_e[:, qoff:512],
                            start=(j == 0),
                            stop=(j == nj - 1),
                        )
                        nc.tensor.matmul(
                            ot_o[:, qoff:512],
                            vp_sb[j][:, 2 * p + 1 : 2 * p + 2, :],
                            p_o[:, qoff:512],
                            start=(j == 0),
                            stop=(j == nj - 1),
                        )

                    def norm():
                        # 1/den via exp(-ln(den)) on ACT (DVE reciprocal is
                        # ~3.5us for a 1-partition row; ACT is ~0.9us)
                        for par, ot_ps in ((0, ot_e), (1, ot_o)):
                            lnd = pInv.tile(
                                [1, 512], DT.float32, tag=f"ln{par}", name="lnd"
                            )
                            nc.scalar.activation(lnd, ot_ps[64:65, :], AF.Ln)
                            inv = pInv.tile(
                                [1, 512], DT.bfloat16, tag=f"iv{par}", name="inv"
                            )
                            nc.scalar.activation(inv, lnd, AF.Exp, scale=-1.0)
                            ibp = psST.tile([128, 512], DT.float32, tag="se", name="ibp")
                            nc.tensor.matmul(
                                ibp[0:64, :],
                                ones_row[0:1, 0:64],
                                inv,
                                start=True,
                                stop=True,
                            )
                            invb = pInv.tile(
                                [64, 512], DT.float32, tag=f"ib{par}", name="invb"
                            )
                            nc.scalar.activation(invb, ibp[0:64, :], AF.Copy)
                            nc.vector.tensor_mul(
                                ot_pair[p][
                                    64 * par : 64 * par + 64, b * 512 : (b + 1) * 512
                                ],
                                ot_ps[0:64, :],
                                invb,
                            )

                    return norm

                def emit_proj(ti, n):
                    """y[t-tile ti, n-chunk] = sum over head pairs (row-packed)."""
                    ye = psY.tile([128, 512], DT.float32, tag="ye", name="ye")
                    yo = psY.tile([128, 512], DT.float32, tag="yo", name="yo")
                    for p in range(4):
                        nc.tensor.matmul(
                            ye,
                            ot_pair[p][0:64, ti * 128 : (ti + 1) * 128],
                            wproj_sb[p][0:64, n * 512 : (n + 1) * 512],
                            start=(p == 0),
                            stop=(p == 3),
                        )
                        nc.tensor.matmul(
                            yo,
                            ot_pair[p][64:128, ti * 128 : (ti + 1) * 128],
                            wproj_sb[p][64:128, n * 512 : (n + 1) * 512],
                            start=(p == 0),
                            stop=(p == 3),
                        )
                    # DVE cannot read two PSUM inputs; stage even half via ACT
                    yeb = pY.tile([128, 512], DT.float32, tag="yeb", name="yeb")
                    nc.scalar.activation(yeb, ye, AF.Copy)
                    ysb = pY.tile([128, 512], DT.float32, tag="ysb", name="ysb")
                    nc.vector.tensor_add(ysb, yeb, yo)
                    nc.gpsimd.dma_start(
                        out=y_d[ti * 128 : (ti + 1) * 128, n * 512 : (n + 1) * 512],
                        in_=ysb,
                    )

                pending = None
                for p in range(4):
                    pending = emit_attn(p, 0, pending)
                for p in range(4):
                    pending = emit_attn(p, 1, pending)
                    # interleave first-half projection into second-half attention
                    emit_proj(p, 0)
                    emit_proj(p, 1)
                pending()
                for ti in range(4, 8):
                    for n in range(2):
                        emit_proj(ti, n)

            if dbg:
                with tc.tile_pool(name="dbgo", bufs=2) as dbgo:
                    for p in range(4):
                        dt3 = dbgo.tile([128, T], DT.float32, tag="dt3", name="dt3")
                        nc.vector.tensor_copy(dt3, ot_pair[p])
                        nc.gpsimd.dma_start(out=dot_d[p], in_=dt3)
    return nc


def _make_mask():
    p = np.arange(128)[:, None]
    c = np.arange(128)[None, :]
    return np.where(c >= p, 0.0, -30000.0).astype(BF16)


def prep_inputs(x, scale, w_qkv, w_proj):
    """Per-core input dict list. Core c: batch c//2, head-group c%2."""
    x = np.asarray(x, np.float32)
    scale = np.asarray(scale, np.float32)
    w_qkv = np.asarray(w_qkv, np.float32)
    w_proj = np.asarray(w_proj, np.float32)
    ws = w_qkv * scale[None, :]  # fold RMSNorm scale into the weights
    mask = _make_mask()
    in_maps = []
    for c in range(N_CORES):
        b, g = c // 2, c % 2
        rows = slice(g * FL, (g + 1) * FL)
        wq = ws[0:E][rows]
        wk = ws[E : 2 * E][rows]
        wv = ws[2 * E : 3 * E][rows]
        in_maps.append(
            {
                "xt": np.ascontiguousarray(x[b].T).astype(BF16),
                "wqk": np.ascontiguousarray(np.concatenate([wq, wk], 0).T).astype(BF16),
                "wv": np.ascontiguousarray(wv.T).astype(BF16),
                "wproj": np.ascontiguousarray(w_proj[:, rows].T).astype(BF16),
                "mask": mask,
            }
        )
    return in_maps


_CACHED_NC = None


def kernel(x, scale, w_qkv, w_proj):
    global _CACHED_NC
    if _CACHED_NC is None:
        _CACHED_NC = build_program()
    in_maps = prep_inputs(x, scale, w_qkv, w_proj)
    res = run_bass_kernel_spmd(_CACHED_NC, in_maps, list(range(N_CORES)))
    out = np.zeros((B, T, E), np.float32)
    for c in range(N_CORES):
        out[c // 2] += res.results[c]["y"]
    return out


if __name__ == "__main__":
    rng = np.random.default_rng(0)
    x = rng.standard_normal((B, T, E), dtype=np.float32)
    scale = np.ones(E, np.float32)
    w_qkv = rng.standard_normal((3 * E, E), dtype=np.float32) / 32
    w_proj = rng.standard_normal((E, E), dtype=np.float32) / 32
    y = kernel(x, scale, w_qkv, w_proj)
    print("ran", y.shape, y.dtype, np.abs(y).mean())
